# revision 4
# baseline (speedup 1.0000x reference)
"""Trainium2 Bass kernel for AdaptivePrototypeContrastiveLoss.

Strategy
--------
Host (cheap, O(N*D) bookkeeping):
  * closed-form momentum EMA + LAPACK QR -> new prototypes  [7,256]
  * row-normalize feats, stable-sort rows by label; scatter the sorted
    columns into 7 uniform zero-padded class segments (pitch W = max
    class count rounded up to 16)
  * the whole scalar epilogue (log, threshold, mean) runs on host from
    shipped per-(row,class) exp sums - it is O(N) numpy work

Device (8 NeuronCores, SPMD, no collectives; all O(N^2) work):
  * row-shard: each core owns 8 row-tiles of 128 rows (64 tiles cover
    rows 0..8191); the last 7 rows (prototypes) are handled exactly on
    host (tiny 7xN job)
  * per row-tile: G = rows @ feats^T via PE (fp8-e4m3 DoubleRow, f32
    PSUM, K=256 per instruction, 512-col chunks in 2048-col supertiles)
  * exp() is split across TWO engines per supertile:
      - ACT computes exact exp(A*sim + BIAS) for cols [0:2048-XW]
      - DVE computes a Schraudolph bit-trick exp for cols [2048-XW:]:
        int16(round(psum*a + b)) whose bit pattern IS bf16(2^t); the
        multiplicative bias of the linear-interp approximation is
        zeroed in expectation via a calibrated offset; residual error
        (~1-2% per element, ~0.05% per class sum) is far inside the
        2e-2 budget and only perturbs log(negsum)
    Both write the same flat bf16 scr row, so the reduction below is
    agnostic to which engine produced each column.
  * class-segment sums: batched bf16 fold tree on strided [128,7,w]
    views; DVE does levels 1-2, the otherwise-idle Pool (gpsimd/Q7)
    engine does levels 3-4 + the final TENSOR_REDUCE -> [128,7]
  * the global max subtraction is replaced by the constant M0=12.5 (the
    max only enters through ~1e-8-scale eps terms, verified offline)
  * input DMA is issued from the Pool queue in compute-first order so
    the first row-tile's operands land first
  * per-core output: [128, 64] per-(row-tile, class) exp sums, one DMA
Host: per-row loss, threshold, mean over 8x[128,64] partials.
"""

import ml_dtypes
import numpy as np

import concourse.bass as bass
import concourse.tile as tile
from concourse import mybir
from concourse.bass_utils import run_bass_kernel_spmd

# ---- problem constants (hardcoded per spec) ----
TEMP = 0.08
EPS = 1e-8
GAMMA = 0.99
BETA = 0.5 * (1.0 - GAMMA)
B, D, C = 8192, 256, 7
N = B + C                      # 8199 rows/cols of the score matrix
NCORES = 8
NT = 8                         # full row-tiles per core (8*8*128 = 8192)
ROWS_PER_CORE = NT * 128       # 1024
SUPER = 2048                   # psum supertile width (4 banks, bufs=2)
XW = 608                       # Schraudolph window width per supertile
POOL_L2C = 5                   # classes of fold L2 done on Pool (rest DVE)
M0 = 12.5                      # constant stand-in for the global max
A_SCALE = 0.5 / float(np.float32(TEMP))
BIAS = (0.5 + EPS) / float(np.float32(TEMP)) - M0
PAD_EXP = float(ml_dtypes.bfloat16(np.exp(np.float32(BIAS))))

# Schraudolph constants: exp(A_SCALE*g + BIAS) ~ bf16_bits(round(a*g + b))
# b includes -7.219 (zero-mean calibration for uniform fractional part)
LOG2E_128 = 128.0 / float(np.log(2.0))
SCH_A = LOG2E_128 * A_SCALE
SCH_B = LOG2E_128 * BIAS + 127.0 * 128.0 - 7.219
SCH_PAD = float(
    np.array([int(round(SCH_B))], np.uint16).view(ml_dtypes.bfloat16)[0]
)

USE_POOL = True                # Pool(Q7) takes fold L3,L4 + reduce

F32 = mybir.dt.float32
BF16 = mybir.dt.bfloat16
I16 = mybir.dt.int16
FP8 = mybir.dt.float8e4
FP8NP = mybir.dt.np(mybir.dt.float8e4)
ALU = mybir.AluOpType
ACTF = mybir.ActivationFunctionType


def _geometry(counts_all):
    """Column-layout geometry: uniform class pitch W, supertiles, and the
    per-supertile ACT/Schraudolph split with pad bookkeeping."""
    w_seg = int(-(-int(counts_all.max()) // 16) * 16)
    npad = C * w_seg
    nsup = npad // SUPER
    tailw = npad - nsup * SUPER
    # matmul chunks within a supertile / tail
    def chunks(width):
        out, o = [], 0
        while o < width:
            wch = min(512, width - o)
            out.append((o, wch))
            o += wch
        return out

    # engine ranges in flat pitched column space: list of (lo, hi, engine)
    ranges = []
    for s in range(nsup):
        base = s * SUPER
        ranges.append((base, base + SUPER - XW, "act"))
        ranges.append((base + SUPER - XW, base + SUPER, "sch"))
    if tailw:
        ranges.append((nsup * SUPER, npad, "act"))

    # pad intervals per class and how many fall in sch windows
    pads_sch = np.zeros(C, dtype=np.int64)
    pads_all = w_seg - counts_all
    for c in range(C):
        plo, phi = c * w_seg + int(counts_all[c]), (c + 1) * w_seg
        for lo, hi, eng in ranges:
            if eng == "sch":
                ov = max(0, min(phi, hi) - max(plo, lo))
                pads_sch[c] += ov
    return {
        "W": w_seg, "NPAD": npad, "NSUP": nsup, "TAILW": tailw,
        "SUPER_CHUNKS": chunks(SUPER), "TAIL_CHUNKS": chunks(tailw),
        "pads_all": pads_all, "pads_sch": pads_sch,
    }


def _split_multi_waits(nc):
    """This container's walrus accepts only ONE sync wait per instruction;
    split extra waits into standalone single-wait EventSemaphore insts."""
    n_new = 0
    for func in nc.m.functions:
        for blk in func.blocks:
            new_insts = []
            for inst in blk.instructions:
                si = getattr(inst, "sync_info", None)
                waits = list(si.on_wait) if si and si.on_wait else []
                if len(waits) > 1:
                    for i, w in enumerate(waits[:-1]):
                        n_new += 1
                        ev = mybir.InstEventSemaphore(
                            name=f"{inst.name}-wsplit{i}",
                            engine=inst.engine,
                            ins=[],
                            outs=[],
                            sync_info=mybir.SyncInfo(on_wait=[w], on_update=[]),
                            bass_nofuse=True,
                        )
                        new_insts.append(ev)
                    si.on_wait = [waits[-1]]
                new_insts.append(inst)
            blk.instructions = new_insts
    return n_new


def _host_prep(features, labels, prototypes, momentums):
    features = np.asarray(features, dtype=np.float32)
    labels = np.asarray(labels).astype(np.int64)
    prototypes = np.asarray(prototypes, dtype=np.float32)
    momentums = np.asarray(momentums, dtype=np.float32)

    # ---- prototype update: closed form of the sequential EMA scan ----
    counts_feat = np.bincount(labels, minlength=C)
    rank = np.zeros(B, dtype=np.int64)
    seen = np.zeros(C, dtype=np.int64)
    for i, l in enumerate(labels):
        rank[i] = seen[l]
        seen[l] += 1
    w = BETA * (GAMMA ** (counts_feat[labels] - 1 - rank).astype(np.float64))
    S = np.zeros((C, B))
    S[labels, np.arange(B)] = w
    m_final = S @ features.astype(np.float64)
    wsum = np.bincount(labels, weights=w, minlength=C)
    m_final -= wsum[:, None] * prototypes.astype(np.float64)
    m_final += (GAMMA ** counts_feat.astype(np.float64))[:, None] * momentums.astype(
        np.float64
    )
    target = prototypes.astype(np.float64) + m_final
    q, _ = np.linalg.qr(target.T.astype(np.float32))
    new_protos = q.T.astype(np.float32)

    # ---- normalized, label-sorted gram operands ----
    feats = np.concatenate([features, new_protos], 0)
    labs = np.concatenate([labels, np.arange(C, dtype=np.int64)])
    nrm = np.linalg.norm(feats.astype(np.float64), axis=-1)
    fhat = feats.astype(np.float64) / nrm[:, None]
    perm = np.argsort(labs, kind="stable")
    fs = fhat[perm]
    ls = labs[perm]
    counts_all = np.bincount(ls, minlength=C)          # includes protos
    bounds = np.concatenate([[0], np.cumsum(counts_all)])  # class col ranges
    geom = _geometry(counts_all)
    W, NPAD = geom["W"], geom["NPAD"]

    fs32 = fs.astype(np.float32)
    Y = np.zeros((D, 8), dtype=np.float64)
    for c in range(C):
        Y[:, c] = fs[bounds[c]:bounds[c + 1]].sum(0)

    # columns scattered into uniform zero-padded class segments
    ftcols = np.zeros((NPAD, D), dtype=np.float32)
    for c in range(C):
        cnt = int(counts_all[c])
        ftcols[c * W:c * W + cnt] = fs32[bounds[c]:bounds[c + 1]]
    ft = np.ascontiguousarray(
        ftcols.T.reshape(2, 128, NPAD).transpose(1, 0, 2)
    ).astype(FP8NP)  # [partition, k-half, col] for DoubleRow

    per_core = []
    for core in range(NCORES):
        base = core * ROWS_PER_CORE
        rows_kt = np.ascontiguousarray(
            fs32[base:base + ROWS_PER_CORE].T.reshape(2, 128, ROWS_PER_CORE)
            .transpose(1, 0, 2)
        ).astype(FP8NP)
        per_core.append({"ft": ft, "rows": rows_kt})

    cnt = counts_all[ls] - 1
    selfsim = (fs32.astype(np.float64) ** 2).sum(1)
    # cross-class pad correction, split by which engine produced the pad
    pads_act = geom["pads_all"] - geom["pads_sch"]
    padv = pads_act * PAD_EXP + geom["pads_sch"] * SCH_PAD  # [C] per-class
    crosspad_all = padv.sum() - padv[ls]                    # [N] per-row
    host = {
        "ls": ls, "counts_all": counts_all, "fs": fs, "Y": Y,
        "selfsim": selfsim, "cnt": cnt, "crosspad": crosspad_all,
    }
    return per_core, host, geom


def _build_graph(geom):
    W, NPAD = geom["W"], geom["NPAD"]
    NSUP, TAILW = geom["NSUP"], geom["TAILW"]
    AW = SUPER - XW            # ACT cols per supertile
    nc = bass.Bass()
    ft_d = nc.declare_dram_parameter("ft", [128, 2, NPAD], FP8, isOutput=False)
    rows_d = nc.declare_dram_parameter(
        "rows", [128, 2, ROWS_PER_CORE], FP8, isOutput=False
    )
    out_d = nc.declare_dram_parameter("out", [128, 64], F32, isOutput=True)

    with tile.TileContext(nc) as tc:
        with (
            tc.tile_pool(name="persist", bufs=1) as persist,
            tc.tile_pool(name="ps", bufs=2, space="PSUM") as psA,
        ):
            # --- resident inputs; DMA ordered so tile 0 operands land first
            rows_sb = persist.tile([128, 2, ROWS_PER_CORE], FP8, tag="rows")
            ft_sb = persist.tile([128, 2, NPAD], FP8, tag="ft")
            nc.gpsimd.dma_start(out=rows_sb[:, :, 0:128], in_=rows_d[:, :, 0:128])
            nc.gpsimd.dma_start(out=ft_sb[:, :, 0:512], in_=ft_d[:, :, 0:512])
            nc.gpsimd.dma_start(out=ft_sb[:, :, 512:2048], in_=ft_d[:, :, 512:2048])
            nc.gpsimd.dma_start(
                out=rows_sb[:, :, 128:1024], in_=rows_d[:, :, 128:1024]
            )
            for o in range(2048, NPAD, 2048):
                hi = min(o + 2048, NPAD)
                nc.gpsimd.dma_start(out=ft_sb[:, :, o:hi], in_=ft_d[:, :, o:hi])

            bias_exp = persist.tile([128, 1], F32, tag="bias_exp")
            nc.vector.memset(bias_exp[:], float(BIAS))
            outbuf = persist.tile([128, 64], F32, tag="outbuf")
            nc.vector.memset(outbuf[:], 0.0)

            scr_a = persist.tile([128, NPAD], BF16, tag="scr0")
            scr_b = persist.tile([128, NPAD], BF16, tag="scr1")
            scr_c = persist.tile([128, NPAD], BF16, tag="scr2")
            scrs = [scr_a, scr_b, scr_c]
            f1 = persist.tile([128, C, W // 2], BF16, tag="f1")
            f2 = persist.tile([128, C, W // 4], BF16, tag="f2")
            f3 = persist.tile([128, C, W // 8], BF16, tag="f3")
            f4 = persist.tile([128, C, W // 16], BF16, tag="f4")

            # --- main loop over row-tiles ---
            for t in range(NT):
                scr = scrs[t % 3]
                lhs_t = rows_sb[:, :, t * 128:(t + 1) * 128]
                for s in range(NSUP):
                    lo = s * SUPER
                    ps = psA.tile([128, SUPER], F32, tag="ps")
                    for o, wch in geom["SUPER_CHUNKS"]:
                        nc.tensor.matmul(
                            ps[:, o:o + wch],
                            lhsT=lhs_t,
                            rhs=ft_sb[:, :, lo + o:lo + o + wch],
                            start=True,
                            stop=True,
                            perf_mode=mybir.MatmulPerfMode.DoubleRow,
                        )
                    # exact exp on ACT for the first AW cols
                    nc.scalar.activation(
                        scr[:, lo:lo + AW],
                        ps[:, 0:AW],
                        ACTF.Exp,
                        bias=bias_exp[:],
                        scale=float(A_SCALE),
                    )
                    # Schraudolph bit-trick exp on DVE for the last XW cols
                    nc.vector.tensor_scalar(
                        out=scr[:, lo + AW:lo + SUPER].bitcast(I16),
                        in0=ps[:, AW:SUPER],
                        scalar1=float(SCH_A),
                        scalar2=float(SCH_B),
                        op0=ALU.mult,
                        op1=ALU.add,
                    )
                # tail: remaining cols, ACT only
                if TAILW:
                    pst = psA.tile([128, SUPER], F32, tag="ps")
                    for o, wch in geom["TAIL_CHUNKS"]:
                        nc.tensor.matmul(
                            pst[:, o:o + wch],
                            lhsT=lhs_t,
                            rhs=ft_sb[:, :, NSUP * SUPER + o:NSUP * SUPER + o + wch],
                            start=True,
                            stop=True,
                            perf_mode=mybir.MatmulPerfMode.DoubleRow,
                        )
                    nc.scalar.activation(
                        scr[:, NSUP * SUPER:NPAD],
                        pst[:, 0:TAILW],
                        ACTF.Exp,
                        bias=bias_exp[:],
                        scale=float(A_SCALE),
                    )

                # class-segment sums: batched bf16 fold tree. DVE does L1,
                # part of L2, and the final reduce; the otherwise-idle Pool
                # (Q7) engine does the rest of L2 plus L3+L4. (gpsimd
                # tensor_reduce only supports partition-axis reductions, so
                # the X-axis reduce must stay on DVE.)
                s3 = scr[:].rearrange("p (c w) -> p c w", c=C)
                h = W // 2
                eng2 = nc.gpsimd if USE_POOL else nc.vector
                pc = POOL_L2C if USE_POOL else 0
                nc.vector.tensor_tensor(
                    out=f1[:], in0=s3[:, :, 0:h], in1=s3[:, :, h:W],
                    op=ALU.add,
                )
                if pc:
                    eng2.tensor_tensor(
                        out=f2[:, 0:pc], in0=f1[:, 0:pc, 0:h // 2],
                        in1=f1[:, 0:pc, h // 2:h], op=ALU.add,
                    )
                if pc < C:
                    nc.vector.tensor_tensor(
                        out=f2[:, pc:C], in0=f1[:, pc:C, 0:h // 2],
                        in1=f1[:, pc:C, h // 2:h], op=ALU.add,
                    )
                eng2.tensor_tensor(
                    out=f3[:], in0=f2[:, :, 0:h // 4],
                    in1=f2[:, :, h // 4:h // 2],
                    op=ALU.add,
                )
                eng2.tensor_tensor(
                    out=f4[:], in0=f3[:, :, 0:h // 8],
                    in1=f3[:, :, h // 8:h // 4],
                    op=ALU.add,
                )
                nc.vector.reduce_sum(
                    outbuf[:, t * 8:t * 8 + C], f4[:], mybir.AxisListType.X
                )

            nc.sync.dma_start(out=out_d[:], in_=outbuf[:])
    return nc


def _combine(results, host):
    """Host-side epilogue: per-row loss from shipped class sums."""
    ls = host["ls"]
    fs, Y = host["fs"], host["Y"]
    selfsim, cnt, crosspad = host["selfsim"], host["cnt"], host["crosspad"]

    loss_sum = 0.0
    cnt_sum = 0.0
    for core in range(NCORES):
        o = np.asarray(results[core]["out"], dtype=np.float64)
        slots = o[:, 0:64].reshape(128, NT, 8)     # [p, t, class]
        base = core * ROWS_PER_CORE
        g = base + np.arange(NT)[None, :] * 128 + np.arange(128)[:, None]
        own = ls[g]                                 # [p, t]
        stot = slots[:, :, 0:C].sum(-1)
        sown = np.take_along_axis(slots, own[:, :, None], axis=2)[:, :, 0]
        negsum = stot - sown - crosspad[g]
        neg = np.log(negsum + EPS)
        possel = np.einsum("ptd,dpt->pt", fs[g], Y[:, own])
        pos = (A_SCALE * (possel - selfsim[g]) + BIAS * cnt[g]) / (cnt[g] + EPS)
        loss = neg - pos
        m = loss > 0
        loss_sum += loss[m].sum()
        cnt_sum += m.sum()

    # rows 8192..8198 (prototypes): tiny 7xN job, computed exactly here
    n7 = N - B  # 7
    sim7 = fs[B:N] @ fs.T                              # [7, N] float64
    E7 = np.exp(A_SCALE * sim7 + BIAS)
    classsum = np.zeros((n7, C), dtype=np.float64)
    for c in range(C):
        classsum[:, c] = E7[:, ls == c].sum(1)
    stot = classsum.sum(1)
    rows_ls = ls[B:N]
    sown = classsum[np.arange(n7), rows_ls]
    neg = np.log(stot - sown + EPS)
    pos_sel = np.einsum("id,di->i", fs[B:N], Y[:, rows_ls])
    pos = (A_SCALE * (pos_sel - host["selfsim"][B:N]) + BIAS * host["cnt"][B:N]) / (
        host["cnt"][B:N] + EPS
    )
    loss64 = -pos + neg
    m = loss64 > 0
    loss_sum += loss64[m].sum()
    cnt_sum += m.sum()

    val = loss_sum / max(cnt_sum, 1.0) if cnt_sum > 0 else 0.0
    return np.float32(val)


def _run(features, labels, prototypes, momentums, trace=False, trace_kwargs=None):
    per_core, host, geom = _host_prep(features, labels, prototypes, momentums)
    nc = _build_graph(geom)
    _split_multi_waits(nc)
    in_maps = [per_core[i] for i in range(NCORES)]
    kw = {}
    if trace:
        kw = dict(trace=True, trace_cores=list(range(NCORES)))
        if trace_kwargs:
            kw["trace_kwargs"] = trace_kwargs
    res = run_bass_kernel_spmd(nc, in_maps, core_ids=list(range(NCORES)), **kw)
    return _combine(res.results, host), res


def kernel(features, labels, prototypes, momentums):
    val, _ = _run(features, labels, prototypes, momentums)
    return np.array(val, dtype=np.float32)


# revision 7
# speedup vs baseline: 1.2716x; 1.2716x over previous
"""Trainium2 Bass kernel for AdaptivePrototypeContrastiveLoss.

Strategy
--------
Host (cheap, O(N*D) bookkeeping):
  * closed-form momentum EMA + LAPACK QR -> new prototypes  [7,256]
  * row-normalize feats, stable-sort rows by label; scatter the sorted
    columns into 7 uniform zero-padded class segments (pitch W = max
    class count rounded up to 16)
  * the whole scalar epilogue (log, threshold, mean) runs on host from
    shipped per-(row,class) exp sums - it is O(N) numpy work

Device (8 NeuronCores, SPMD, no collectives; all O(N^2) work):
  * row-shard: each core owns 8 row-tiles of 128 rows (64 tiles cover
    rows 0..8191); the last 7 rows (prototypes) are handled exactly on
    host (tiny 7xN job)
  * per row-tile: G = rows @ feats^T via PE (fp8-e4m3 DoubleRow, f32
    PSUM, K=256 per instruction, 512-col chunks in 2048-col supertiles)
  * exp() is split across TWO engines per supertile:
      - ACT computes exact exp(A*sim + BIAS) for cols [0:2048-XW]
      - DVE computes a Schraudolph bit-trick exp for cols [2048-XW:]:
        int16(round(psum*a + b)) whose bit pattern IS bf16(2^t); the
        multiplicative bias of the linear-interp approximation is
        zeroed in expectation via a calibrated offset; residual error
        only perturbs log(negsum) and is ~100x inside the 2e-2 budget
    Both write the same flat bf16 scr row, so the reduction below is
    agnostic to which engine produced each column.
  * class-segment sums: batched bf16 fold tree on strided [128,7,w]
    views. The fold work for tile t is SOFTWARE-PIPELINED one tile
    behind compute: DVE does fold levels 1-2 of tile t-1 between the
    Schraudolph windows of tile t, the otherwise-idle Pool (gpsimd/Q7)
    engine does levels 3-5 of tile t-1, and DVE finishes with the
    TENSOR_REDUCE of tile t-2. All fold scratch (f1..f5) and scr are
    double-buffered so consecutive tiles never serialize on WAR deps.
  * the global max subtraction is replaced by the constant M0=12.5 (the
    max only enters through ~1e-8-scale eps terms, verified offline)
  * the ACT exp table load is hoisted to t=0 (dummy activation) so the
    first real exp doesn't eat the ~2.7us table-load latency
  * input DMA is issued from both the Pool and Sync queues in
    compute-first order so the first row-tile's operands land first
  * per-core output: [128, 64] per-(row-tile, class) exp sums, one DMA
Host: per-row loss, threshold, mean over 8x[128,64] partials.
"""

import ml_dtypes
import numpy as np

import concourse.bass as bass
import concourse.tile as tile
from concourse import mybir
from concourse.bass_utils import run_bass_kernel_spmd

# ---- problem constants (hardcoded per spec) ----
TEMP = 0.08
EPS = 1e-8
GAMMA = 0.99
BETA = 0.5 * (1.0 - GAMMA)
B, D, C = 8192, 256, 7
N = B + C                      # 8199 rows/cols of the score matrix
NCORES = 8
NT = 8                         # full row-tiles per core (8*8*128 = 8192)
ROWS_PER_CORE = NT * 128       # 1024
SUPER = 2048                   # psum supertile width (4 banks, bufs=2)
XW = 416                       # Schraudolph window width per supertile
M0 = 12.5                      # constant stand-in for the global max
A_SCALE = 0.5 / float(np.float32(TEMP))
BIAS = (0.5 + EPS) / float(np.float32(TEMP)) - M0
PAD_EXP = float(ml_dtypes.bfloat16(np.exp(np.float32(BIAS))))

# Schraudolph constants: exp(A_SCALE*g + BIAS) ~ bf16_bits(round(a*g + b))
# b includes -7.219 (zero-mean calibration for uniform fractional part)
LOG2E_128 = 128.0 / float(np.log(2.0))
SCH_A = LOG2E_128 * A_SCALE
SCH_B = LOG2E_128 * BIAS + 127.0 * 128.0 - 7.219
SCH_PAD = float(
    np.array([int(round(SCH_B))], np.uint16).view(ml_dtypes.bfloat16)[0]
)

F32 = mybir.dt.float32
BF16 = mybir.dt.bfloat16
I16 = mybir.dt.int16
FP8 = mybir.dt.float8e4
FP8NP = mybir.dt.np(mybir.dt.float8e4)
ALU = mybir.AluOpType
ACTF = mybir.ActivationFunctionType


def _geometry(counts_all):
    """Column-layout geometry: uniform class pitch W, supertiles, and the
    per-supertile ACT/Schraudolph split with pad bookkeeping."""
    w_seg = int(-(-int(counts_all.max()) // 16) * 16)
    npad = C * w_seg
    nsup = npad // SUPER
    tailw = npad - nsup * SUPER

    def chunks(width):
        out, o = [], 0
        while o < width:
            wch = min(512, width - o)
            out.append((o, wch))
            o += wch
        return out

    # engine ranges in flat pitched column space: list of (lo, hi, engine)
    ranges = []
    for s in range(nsup):
        base = s * SUPER
        ranges.append((base, base + SUPER - XW, "act"))
        ranges.append((base + SUPER - XW, base + SUPER, "sch"))
    if tailw:
        ranges.append((nsup * SUPER, npad, "act"))

    pads_sch = np.zeros(C, dtype=np.int64)
    pads_all = w_seg - counts_all
    for c in range(C):
        plo, phi = c * w_seg + int(counts_all[c]), (c + 1) * w_seg
        for lo, hi, eng in ranges:
            if eng == "sch":
                ov = max(0, min(phi, hi) - max(plo, lo))
                pads_sch[c] += ov
    return {
        "W": w_seg, "NPAD": npad, "NSUP": nsup, "TAILW": tailw,
        "SUPER_CHUNKS": chunks(SUPER), "TAIL_CHUNKS": chunks(tailw),
        "pads_all": pads_all, "pads_sch": pads_sch,
    }


def _split_multi_waits(nc):
    """This container's walrus accepts only ONE sync wait per instruction;
    split extra waits into standalone single-wait EventSemaphore insts."""
    n_new = 0
    for func in nc.m.functions:
        for blk in func.blocks:
            new_insts = []
            for inst in blk.instructions:
                si = getattr(inst, "sync_info", None)
                waits = list(si.on_wait) if si and si.on_wait else []
                if len(waits) > 1:
                    for i, w in enumerate(waits[:-1]):
                        n_new += 1
                        ev = mybir.InstEventSemaphore(
                            name=f"{inst.name}-wsplit{i}",
                            engine=inst.engine,
                            ins=[],
                            outs=[],
                            sync_info=mybir.SyncInfo(on_wait=[w], on_update=[]),
                            bass_nofuse=True,
                        )
                        new_insts.append(ev)
                    si.on_wait = [waits[-1]]
                new_insts.append(inst)
            blk.instructions = new_insts
    return n_new


def _host_prep(features, labels, prototypes, momentums):
    features = np.asarray(features, dtype=np.float32)
    labels = np.asarray(labels).astype(np.int64)
    prototypes = np.asarray(prototypes, dtype=np.float32)
    momentums = np.asarray(momentums, dtype=np.float32)

    # ---- prototype update: closed form of the sequential EMA scan ----
    counts_feat = np.bincount(labels, minlength=C)
    rank = np.zeros(B, dtype=np.int64)
    seen = np.zeros(C, dtype=np.int64)
    for i, l in enumerate(labels):
        rank[i] = seen[l]
        seen[l] += 1
    w = BETA * (GAMMA ** (counts_feat[labels] - 1 - rank).astype(np.float64))
    S = np.zeros((C, B))
    S[labels, np.arange(B)] = w
    m_final = S @ features.astype(np.float64)
    wsum = np.bincount(labels, weights=w, minlength=C)
    m_final -= wsum[:, None] * prototypes.astype(np.float64)
    m_final += (GAMMA ** counts_feat.astype(np.float64))[:, None] * momentums.astype(
        np.float64
    )
    target = prototypes.astype(np.float64) + m_final
    q, _ = np.linalg.qr(target.T.astype(np.float32))
    new_protos = q.T.astype(np.float32)

    # ---- normalized, label-sorted gram operands ----
    feats = np.concatenate([features, new_protos], 0)
    labs = np.concatenate([labels, np.arange(C, dtype=np.int64)])
    nrm = np.linalg.norm(feats.astype(np.float64), axis=-1)
    fhat = feats.astype(np.float64) / nrm[:, None]
    perm = np.argsort(labs, kind="stable")
    fs = fhat[perm]
    ls = labs[perm]
    counts_all = np.bincount(ls, minlength=C)          # includes protos
    bounds = np.concatenate([[0], np.cumsum(counts_all)])  # class col ranges
    geom = _geometry(counts_all)
    W, NPAD = geom["W"], geom["NPAD"]

    fs32 = fs.astype(np.float32)
    Y = np.zeros((D, 8), dtype=np.float64)
    for c in range(C):
        Y[:, c] = fs[bounds[c]:bounds[c + 1]].sum(0)

    # columns scattered into uniform zero-padded class segments
    ftcols = np.zeros((NPAD, D), dtype=np.float32)
    for c in range(C):
        cnt = int(counts_all[c])
        ftcols[c * W:c * W + cnt] = fs32[bounds[c]:bounds[c + 1]]
    ft = np.ascontiguousarray(
        ftcols.T.reshape(2, 128, NPAD).transpose(1, 0, 2)
    ).astype(FP8NP)  # [partition, k-half, col] for DoubleRow

    per_core = []
    for core in range(NCORES):
        base = core * ROWS_PER_CORE
        rows_kt = np.ascontiguousarray(
            fs32[base:base + ROWS_PER_CORE].T.reshape(2, 128, ROWS_PER_CORE)
            .transpose(1, 0, 2)
        ).astype(FP8NP)
        per_core.append({"ft": ft, "rows": rows_kt})

    cnt = counts_all[ls] - 1
    selfsim = (fs32.astype(np.float64) ** 2).sum(1)
    # cross-class pad correction, split by which engine produced the pad
    pads_act = geom["pads_all"] - geom["pads_sch"]
    padv = pads_act * PAD_EXP + geom["pads_sch"] * SCH_PAD  # [C] per-class
    crosspad_all = padv.sum() - padv[ls]                    # [N] per-row
    host = {
        "ls": ls, "counts_all": counts_all, "fs": fs, "Y": Y,
        "selfsim": selfsim, "cnt": cnt, "crosspad": crosspad_all,
    }
    return per_core, host, geom


def _build_graph(geom):
    W, NPAD = geom["W"], geom["NPAD"]
    NSUP, TAILW = geom["NSUP"], geom["TAILW"]
    AW = SUPER - XW            # ACT cols per supertile
    nc = bass.Bass()
    ft_d = nc.declare_dram_parameter("ft", [128, 2, NPAD], FP8, isOutput=False)
    rows_d = nc.declare_dram_parameter(
        "rows", [128, 2, ROWS_PER_CORE], FP8, isOutput=False
    )
    out_d = nc.declare_dram_parameter("out", [128, 64], F32, isOutput=True)

    with tile.TileContext(nc) as tc:
        with (
            tc.tile_pool(name="persist", bufs=1) as persist,
            tc.tile_pool(name="ps", bufs=2, space="PSUM") as psA,
        ):
            # --- resident inputs; DMA ordered so tile 0 operands land first,
            # split across two issue queues so DGE setup overlaps
            rows_sb = persist.tile([128, 2, ROWS_PER_CORE], FP8, tag="rows")
            ft_sb = persist.tile([128, 2, NPAD], FP8, tag="ft")
            nc.sync.dma_start(out=rows_sb[:, :, 0:128], in_=rows_d[:, :, 0:128])
            nc.gpsimd.dma_start(out=ft_sb[:, :, 0:512], in_=ft_d[:, :, 0:512])
            nc.gpsimd.dma_start(out=ft_sb[:, :, 512:2048], in_=ft_d[:, :, 512:2048])
            nc.sync.dma_start(
                out=rows_sb[:, :, 128:1024], in_=rows_d[:, :, 128:1024]
            )
            for i, o in enumerate(range(2048, NPAD, 2048)):
                hi = min(o + 2048, NPAD)
                eng = nc.gpsimd if i % 2 == 0 else nc.sync
                eng.dma_start(out=ft_sb[:, :, o:hi], in_=ft_d[:, :, o:hi])

            outbuf = persist.tile([128, 64], F32, tag="outbuf")
            nc.vector.memset(outbuf[:], 0.0)
            bias_exp = persist.tile([128, 1], F32, tag="bias_exp")
            nc.vector.memset(bias_exp[:], float(BIAS))
            # hoist the exp table load: dummy 1-col activation at t=0
            dummy = persist.tile([128, 1], F32, tag="dummy")
            nc.scalar.activation(dummy[:], outbuf[:, 0:1], ACTF.Exp)

            scr_a = persist.tile([128, NPAD], BF16, tag="scr0")
            scr_b = persist.tile([128, NPAD], BF16, tag="scr1")
            scrs = [scr_a, scr_b]
            # double-buffered fold scratch
            fbufs = []
            for p in range(2):
                fb = {}
                for lvl in range(1, 6):
                    fb[lvl] = persist.tile(
                        [128, C, W // (2 ** lvl)], BF16,
                        tag=f"f{lvl}_{p}", name=f"f{lvl}_{p}",
                    )
                fbufs.append(fb)

            def compute_tile(t, dve_slots):
                """Emit matmuls+ACT for tile t; interleave dve_slots (list of
                callables emitting DVE fold work of earlier tiles) between
                the Schraudolph windows."""
                scr = scrs[t % 2]
                lhs_t = rows_sb[:, :, t * 128:(t + 1) * 128]
                slot = 0
                for s in range(NSUP):
                    lo = s * SUPER
                    ps = psA.tile([128, SUPER], F32, tag="ps")
                    for o, wch in geom["SUPER_CHUNKS"]:
                        nc.tensor.matmul(
                            ps[:, o:o + wch],
                            lhsT=lhs_t,
                            rhs=ft_sb[:, :, lo + o:lo + o + wch],
                            start=True,
                            stop=True,
                            perf_mode=mybir.MatmulPerfMode.DoubleRow,
                        )
                    nc.scalar.activation(
                        scr[:, lo:lo + AW],
                        ps[:, 0:AW],
                        ACTF.Exp,
                        bias=bias_exp[:],
                        scale=float(A_SCALE),
                    )
                    nc.vector.tensor_scalar(
                        out=scr[:, lo + AW:lo + SUPER].bitcast(I16),
                        in0=ps[:, AW:SUPER],
                        scalar1=float(SCH_A),
                        scalar2=float(SCH_B),
                        op0=ALU.mult,
                        op1=ALU.add,
                    )
                    # interleave one pending DVE fold stage after each window
                    if s >= 1 and slot < len(dve_slots):
                        dve_slots[slot]()
                        slot += 1
                if TAILW:
                    pst = psA.tile([128, SUPER], F32, tag="ps")
                    for o, wch in geom["TAIL_CHUNKS"]:
                        nc.tensor.matmul(
                            pst[:, o:o + wch],
                            lhsT=lhs_t,
                            rhs=ft_sb[:, :, NSUP * SUPER + o:NSUP * SUPER + o + wch],
                            start=True,
                            stop=True,
                            perf_mode=mybir.MatmulPerfMode.DoubleRow,
                        )
                    nc.scalar.activation(
                        scr[:, NSUP * SUPER:NPAD],
                        pst[:, 0:TAILW],
                        ACTF.Exp,
                        bias=bias_exp[:],
                        scale=float(A_SCALE),
                    )
                while slot < len(dve_slots):
                    dve_slots[slot]()
                    slot += 1

            def s1_l1(t):
                fb = fbufs[t % 2]
                s3 = scrs[t % 2][:].rearrange("p (c w) -> p c w", c=C)
                h = W // 2
                nc.vector.tensor_tensor(
                    out=fb[1][:], in0=s3[:, :, 0:h], in1=s3[:, :, h:W],
                    op=ALU.add,
                )

            def s1_l2(t):
                fb = fbufs[t % 2]
                h2 = W // 4
                nc.vector.tensor_tensor(
                    out=fb[2][:], in0=fb[1][:, :, 0:h2], in1=fb[1][:, :, h2:2 * h2],
                    op=ALU.add,
                )

            def s2_pool(t):
                fb = fbufs[t % 2]
                h3, h4, h5 = W // 8, W // 16, W // 32
                nc.gpsimd.tensor_tensor(
                    out=fb[3][:], in0=fb[2][:, :, 0:h3], in1=fb[2][:, :, h3:2 * h3],
                    op=ALU.add,
                )
                nc.gpsimd.tensor_tensor(
                    out=fb[4][:], in0=fb[3][:, :, 0:h4], in1=fb[3][:, :, h4:2 * h4],
                    op=ALU.add,
                )
                nc.gpsimd.tensor_tensor(
                    out=fb[5][:], in0=fb[4][:, :, 0:h5], in1=fb[4][:, :, h5:2 * h5],
                    op=ALU.add,
                )

            def s3_red(t):
                fb = fbufs[t % 2]
                nc.vector.reduce_sum(
                    outbuf[:, t * 8:t * 8 + C], fb[5][:], mybir.AxisListType.X
                )

            # --- software-pipelined main loop ---
            for t in range(NT):
                slots = []
                if t >= 1:
                    slots.append(lambda tt=t - 1: s1_l1(tt))
                    slots.append(lambda tt=t - 1: s1_l2(tt))
                if t >= 2:
                    slots.append(lambda tt=t - 2: s3_red(tt))
                compute_tile(t, slots)
                if t >= 1:
                    s2_pool(t - 1)
            # drain: folds of the last tile + last reduces
            s1_l1(NT - 1)
            s1_l2(NT - 1)
            s3_red(NT - 2)
            s2_pool(NT - 1)
            s3_red(NT - 1)

            nc.sync.dma_start(out=out_d[:], in_=outbuf[:])
    return nc


def _combine(results, host):
    """Host-side epilogue: per-row loss from shipped class sums."""
    ls = host["ls"]
    fs, Y = host["fs"], host["Y"]
    selfsim, cnt, crosspad = host["selfsim"], host["cnt"], host["crosspad"]

    loss_sum = 0.0
    cnt_sum = 0.0
    for core in range(NCORES):
        o = np.asarray(results[core]["out"], dtype=np.float64)
        slots = o[:, 0:64].reshape(128, NT, 8)     # [p, t, class]
        base = core * ROWS_PER_CORE
        g = base + np.arange(NT)[None, :] * 128 + np.arange(128)[:, None]
        own = ls[g]                                 # [p, t]
        stot = slots[:, :, 0:C].sum(-1)
        sown = np.take_along_axis(slots, own[:, :, None], axis=2)[:, :, 0]
        negsum = stot - sown - crosspad[g]
        neg = np.log(negsum + EPS)
        possel = np.einsum("ptd,dpt->pt", fs[g], Y[:, own])
        pos = (A_SCALE * (possel - selfsim[g]) + BIAS * cnt[g]) / (cnt[g] + EPS)
        loss = neg - pos
        m = loss > 0
        loss_sum += loss[m].sum()
        cnt_sum += m.sum()

    # rows 8192..8198 (prototypes): tiny 7xN job, computed exactly here
    n7 = N - B  # 7
    sim7 = fs[B:N] @ fs.T                              # [7, N] float64
    E7 = np.exp(A_SCALE * sim7 + BIAS)
    classsum = np.zeros((n7, C), dtype=np.float64)
    for c in range(C):
        classsum[:, c] = E7[:, ls == c].sum(1)
    stot = classsum.sum(1)
    rows_ls = ls[B:N]
    sown = classsum[np.arange(n7), rows_ls]
    neg = np.log(stot - sown + EPS)
    pos_sel = np.einsum("id,di->i", fs[B:N], Y[:, rows_ls])
    pos = (A_SCALE * (pos_sel - host["selfsim"][B:N]) + BIAS * host["cnt"][B:N]) / (
        host["cnt"][B:N] + EPS
    )
    loss64 = -pos + neg
    m = loss64 > 0
    loss_sum += loss64[m].sum()
    cnt_sum += m.sum()

    val = loss_sum / max(cnt_sum, 1.0) if cnt_sum > 0 else 0.0
    return np.float32(val)


def _run(features, labels, prototypes, momentums, trace=False, trace_kwargs=None):
    per_core, host, geom = _host_prep(features, labels, prototypes, momentums)
    nc = _build_graph(geom)
    _split_multi_waits(nc)
    in_maps = [per_core[i] for i in range(NCORES)]
    kw = {}
    if trace:
        kw = dict(trace=True, trace_cores=list(range(NCORES)))
        if trace_kwargs:
            kw["trace_kwargs"] = trace_kwargs
    res = run_bass_kernel_spmd(nc, in_maps, core_ids=list(range(NCORES)), **kw)
    return _combine(res.results, host), res


def kernel(features, labels, prototypes, momentums):
    val, _ = _run(features, labels, prototypes, momentums)
    return np.array(val, dtype=np.float32)
